# revision 3
# baseline (speedup 1.0000x reference)
"""Trainium2 Bass kernel for a fused CustomLSTMCell.

Math (reference):
    combined = concat([input, hidden], axis=1)            # [B, D], D = 2048
    gates    = combined @ concat([W_i,W_f,W_o,W_g], 1) + b  # [B, 4H]
    i, f, o, g = split(gates, 4, axis=1)
    new_cell   = sigmoid(f) * cell_state + sigmoid(i) * tanh(g)
    new_hidden = sigmoid(o) * tanh(new_cell)

Strategy:
  - Data-parallel over batch: 8 cores x 1024 rows each. No collectives.
  - Host prepares A^T = combined.T (contraction dim D on partitions) in bf16
    and W in bf16; PSUM accumulates in fp32.
  - Per core GEMM: [1024, 2048] @ [2048, 4096] as 128x128x512 matmul tiles.
  - Loop order: gate-column group OUTER (2 groups of 512 cols per gate),
    batch m-tile inner. Each group's weight slice (8 MB) is consumed over
    ~half the iteration, so weight DMA (paced at ~150 GB/s) stays ahead of
    the PE instead of starving it at iteration start.
  - For_i back edge uses staggered_reset so next-iteration input DMAs overlap
    tail compute instead of a full all-engine barrier.
  - Elementwise LSTM math in bf16 (2x DVE) on [128 x 512] tiles; cell-state
    in / new_hidden+new_cell out are bf16 to halve that HBM traffic.
"""

import sys

if "/opt/trn_rl_repo" not in sys.path:
    sys.path.insert(0, "/opt/trn_rl_repo")

import ml_dtypes
import numpy as np

import concourse.bass as bass
import concourse.mybir as mybir
import concourse.tile as tile
from concourse import bacc
from concourse.bass_utils import run_bass_kernel_spmd

N_CORES = 8
B = 8192
IN_SIZE = 1024
H = 1024
D = IN_SIZE + H          # 2048 contraction dim
G4 = 4 * H               # 4096 gate columns
BC = B // N_CORES        # 1024 batch rows per core
P = 128                  # partitions
KT = D // P              # 16 k-tiles
MT = BC // P             # 8 m-tiles (batch row tiles) per core
NG = 512                 # gate columns processed per group (per gate)
HGRPS = H // NG          # 2 column groups

_NC_CACHE = {}


def _build(iters: int = 1, loads_in_loop: bool = True, compute=True,
           staggered: bool = True) -> bass.Bass:
    # compute: True = full body, False = no compute, "mm" = matmuls only
    nc = bacc.Bacc("TRN2", target_bir_lowering=False, debug=False)

    at = nc.dram_tensor("at", [D, BC], mybir.dt.bfloat16, kind="ExternalInput")
    w = nc.dram_tensor("w", [D, G4], mybir.dt.bfloat16, kind="ExternalInput")
    br = nc.dram_tensor("br", [P, G4], mybir.dt.float32, kind="ExternalInput")
    cs = nc.dram_tensor("cs", [BC, H], mybir.dt.bfloat16, kind="ExternalInput")
    nh = nc.dram_tensor("nh", [BC, H], mybir.dt.bfloat16, kind="ExternalOutput")
    ncl = nc.dram_tensor("ncl", [BC, H], mybir.dt.bfloat16, kind="ExternalOutput")

    at_r = at.rearrange("(ko ki) b -> ki ko b", ki=P)        # [128, KT, BC]
    w4 = w.rearrange("(ko ki) (q n) -> ki ko q n", ki=P, q=4)  # [128, KT, 4, H]
    cs_r = cs.rearrange("(m p) h -> m p h", p=P)             # [MT, 128, H]
    nh_r = nh.rearrange("(m p) h -> m p h", p=P)
    ncl_r = ncl.rearrange("(m p) h -> m p h", p=P)

    AF = mybir.ActivationFunctionType

    from contextlib import nullcontext

    with tile.TileContext(nc) as tc:
        with (
            tc.tile_pool(name="resident", bufs=1) as rpool,
            tc.tile_pool(name="work", bufs=3) as wpool,
            tc.tile_pool(name="psum", bufs=2, space="PSUM") as ppool,
        ):
            def loads():
                br_sb = rpool.tile([P, G4], mybir.dt.float32, tag="br", name="br_sb")
                at_sb = rpool.tile([P, KT, BC], mybir.dt.bfloat16, tag="at", name="at_sb")
                w_sb = rpool.tile([P, KT, 4, H], mybir.dt.bfloat16, tag="w", name="w_sb")
                nc.sync.dma_start(out=br_sb[:], in_=br[:])
                # issue in consumption order: (at k, w group-0 k) first, then
                # group-1 weight slices. at is split per m-half so the next
                # iteration's reload can begin as soon as the first half's
                # last reader (an early m-tile) is done.
                for k in range(KT):
                    nc.sync.dma_start(out=at_sb[:, k, 0:512], in_=at_r[:, k, 0:512])
                    nc.sync.dma_start(out=at_sb[:, k, 512:1024], in_=at_r[:, k, 512:1024])
                    nc.sync.dma_start(
                        out=w_sb[:, k, :, 0:NG], in_=w4[:, k, :, 0:NG]
                    )
                for k in range(KT):
                    nc.sync.dma_start(
                        out=w_sb[:, k, :, NG:H], in_=w4[:, k, :, NG:H]
                    )
                return at_sb, w_sb, br_sb

            if not loads_in_loop:
                at_sb, w_sb, br_sb = loads()
            loop = (
                tc.For_i(0, iters, 1, staggered_reset=staggered,
                         hint_engines=(mybir.EngineType.PE,))
                if iters > 1
                else nullcontext()
            )
            with loop:
                if loads_in_loop:
                    at_sb, w_sb, br_sb = loads()
                if compute is not True:
                    # diagnostic variants skip eltwise: still write outputs so
                    # the NEFF has all ExternalOutputs produced
                    dummy = wpool.tile([P, H], mybir.dt.bfloat16, tag="dummy")
                    nc.vector.tensor_copy(out=dummy[:], in_=at_sb[:, 0, 0:H])
                    nc.sync.dma_start(out=nh_r[0, :, :], in_=dummy[:])
                    nc.sync.dma_start(out=ncl_r[0, :, :], in_=dummy[:])
                for g in range(HGRPS if compute else 0):
                    for m in range(MT):
                        ps = [
                            ppool.tile([P, NG], mybir.dt.float32, tag=f"ps{q}", name=f"ps{q}")
                            for q in range(4)
                        ]
                        for k in range(KT):
                            lhs = at_sb[:, k, m * P : (m + 1) * P]
                            for q in range(4):
                                nc.tensor.matmul(
                                    ps[q][:],
                                    lhsT=lhs,
                                    rhs=w_sb[:, k, q, g * NG : (g + 1) * NG],
                                    start=(k == 0),
                                    stop=(k == KT - 1),
                                )
                        if compute == "mm":
                            continue
                        cols = [q * H + g * NG for q in range(4)]
                        # bias add (DVE, PSUM->SBUF bf16) + activation (ACT)
                        gt = [
                            wpool.tile([P, NG], mybir.dt.bfloat16, tag=f"gt{q}", name=f"gt{q}")
                            for q in range(4)
                        ]
                        for q in range(4):
                            nc.vector.tensor_add(
                                out=gt[q][:],
                                in0=ps[q][:],
                                in1=br_sb[:, cols[q] : cols[q] + NG],
                            )
                        for q in range(3):
                            nc.scalar.activation(gt[q][:], gt[q][:], AF.Sigmoid)
                        nc.scalar.activation(gt[3][:], gt[3][:], AF.Tanh)

                        cl = wpool.tile([P, NG], mybir.dt.bfloat16, tag="cl")
                        nc.sync.dma_start(
                            out=cl[:], in_=cs_r[m, :, g * NG : (g + 1) * NG]
                        )
                        si, sf, so, sg = gt
                        # new_cell = sigmoid(f)*c + sigmoid(i)*tanh(g)  -> sf
                        nc.vector.tensor_mul(out=sf[:], in0=sf[:], in1=cl[:])
                        nc.vector.tensor_mul(out=si[:], in0=si[:], in1=sg[:])
                        nc.vector.tensor_add(out=sf[:], in0=sf[:], in1=si[:])
                        # new_hidden = sigmoid(o)*tanh(new_cell)        -> so
                        nc.scalar.activation(sg[:], sf[:], AF.Tanh)
                        nc.vector.tensor_mul(out=so[:], in0=so[:], in1=sg[:])
                        nc.sync.dma_start(
                            out=ncl_r[m, :, g * NG : (g + 1) * NG], in_=sf[:]
                        )
                        nc.sync.dma_start(
                            out=nh_r[m, :, g * NG : (g + 1) * NG], in_=so[:]
                        )
    nc.finalize()
    return nc


def get_nc(iters: int = 1, loads_in_loop: bool = True, compute=True,
           staggered: bool = True) -> bass.Bass:
    key = (iters, loads_in_loop, compute, staggered)
    if key not in _NC_CACHE:
        _NC_CACHE[key] = _build(iters, loads_in_loop, compute, staggered)
    return _NC_CACHE[key]


def make_in_maps(input, hidden, cell_state, W_i, b_i, W_f, b_f, W_o, b_o, W_g, b_g):
    comb = np.concatenate(
        [np.asarray(input, np.float32), np.asarray(hidden, np.float32)], axis=1
    )  # [B, D]
    W = np.concatenate(
        [np.asarray(W_i), np.asarray(W_f), np.asarray(W_o), np.asarray(W_g)], axis=1
    ).astype(np.float32)  # [D, 4H]
    b = np.concatenate(
        [np.asarray(b_i), np.asarray(b_f), np.asarray(b_o), np.asarray(b_g)]
    ).astype(np.float32)  # [4H]

    at_full = comb.T.astype(ml_dtypes.bfloat16)  # [D, B]
    w_bf = np.ascontiguousarray(W.astype(ml_dtypes.bfloat16))
    br = np.ascontiguousarray(np.broadcast_to(b, (P, G4)))
    cs = np.asarray(cell_state, np.float32).astype(ml_dtypes.bfloat16)

    in_maps = []
    for c in range(N_CORES):
        sl = slice(c * BC, (c + 1) * BC)
        in_maps.append(
            {
                "at": np.ascontiguousarray(at_full[:, sl]),
                "w": w_bf,
                "br": br,
                "cs": np.ascontiguousarray(cs[sl]),
            }
        )
    return in_maps


def kernel(input, hidden, cell_state, W_i, b_i, W_f, b_f, W_o, b_o, W_g, b_g):
    in_maps = make_in_maps(
        input, hidden, cell_state, W_i, b_i, W_f, b_f, W_o, b_o, W_g, b_g
    )
    nc = get_nc(1)
    res = run_bass_kernel_spmd(nc, in_maps, core_ids=list(range(N_CORES)))
    new_hidden = np.concatenate(
        [res.results[c]["nh"].astype(np.float32) for c in range(N_CORES)], axis=0
    )
    new_cell = np.concatenate(
        [res.results[c]["ncl"].astype(np.float32) for c in range(N_CORES)], axis=0
    )
    return new_hidden, new_cell
